# revision 8
# baseline (speedup 1.0000x reference)
"""Trainium2 Bass kernel for nn_Cross_attention_multi (sparse_attention).

Pipeline (8 NeuronCores, SPMD, one NEFF):
  Stage A  - 3D conv (SAME, 3x3x3) spatially sharded: each core convolves a
             6-row h-strip for all 32 channels of both x and y. bf16 matmuls
             with K=96 (ci x kh) and M=128 (4 output d-slices x 32 channels)
             accumulate 3 kw-taps per input d-slab into f32 PSUM; the kd tap
             is absorbed into the M-packing (each input slab feeds the 3
             output d's that need it with the right kd weights).
  AllToAll - one collective per tensor (x's overlaps y's conv) redistributes
             conv output (bf16) from spatial shards to channel shards in a
             patch-major layout.
  Stage B  - per channel: gather t^T [81, 1024] (patch dim on partitions),
             res_trans = W1/W2 matmuls + leaky relu(0.2), attention
             a^T.T @ b^T in [128, 512] PSUM tiles, streamed to HBM as f32.
"""

import sys

sys.path.insert(0, "/opt/trn_rl_repo")

import numpy as np
import ml_dtypes

import concourse.bass as bass
import concourse.bacc as bacc
import concourse.mybir as mybir
import concourse.tile as tile
from concourse import bass_utils

N_CORES = 8
C, D, H, W = 32, 36, 48, 48
P = 9
L = 1024
F32 = mybir.dt.float32
BF16 = mybir.dt.bfloat16
BF_NP = ml_dtypes.bfloat16


def build_program(n_iters=1, phases="abc"):
    nc = bacc.Bacc(
        "TRN2", target_bir_lowering=False, debug=False, num_devices=N_CORES
    )

    xs = nc.dram_tensor("xs", [C, D, 8, 50], BF16, kind="ExternalInput")
    ys = nc.dram_tensor("ys", [C, D, 8, 50], BF16, kind="ExternalInput")
    # [rel_d(6), kw(3), kh*32+ci (96), 32*dd+co (128)]
    lwx = nc.dram_tensor("lwx", [6, 3, 96, 128], BF16, kind="ExternalInput")
    lwy = nc.dram_tensor("lwy", [6, 3, 96, 128], BF16, kind="ExternalInput")
    bxv = nc.dram_tensor("bxv", [32, 1], F32, kind="ExternalInput")
    byv = nc.dram_tensor("byv", [32, 1], F32, kind="ExternalInput")
    w1ta = nc.dram_tensor("w1ta", [81, 81], BF16, kind="ExternalInput")
    w1tb = nc.dram_tensor("w1tb", [81, 81], BF16, kind="ExternalInput")
    w2t1 = nc.dram_tensor("w2t1", [81, 81], BF16, kind="ExternalInput")
    w2t2 = nc.dram_tensor("w2t2", [81, 81], BF16, kind="ExternalInput")
    att = nc.dram_tensor("att", [4, L, L], F32, kind="ExternalOutput")

    Ident = mybir.ActivationFunctionType.Identity
    Copy = mybir.ActivationFunctionType.Copy
    mult = mybir.AluOpType.mult
    amax = mybir.AluOpType.max

    with tile.TileContext(nc) as tc:
        # [shard core][c_lo][p81 = pd*9+pw][ld*32 + lhw_local], per tensor
        with tc.tile_pool(name="dram", bufs=1, space="DRAM") as dram:
            a2a_in = [dram.tile([N_CORES, 4, 81, 128], BF16, name=f"a2ai{t}") for t in range(2)]
            a2a_out = [dram.tile([N_CORES, 4, 81, 128], BF16, name=f"a2ao{t}") for t in range(2)]

            for _it in range(n_iters):
                if _it:
                    tc.strict_bb_all_engine_barrier()
                # ------------ Stage A: conv (+ per-tensor AllToAll) ------
                if "a" in phases:
                    with (
                        tc.tile_pool(name="slab", bufs=2) as slab_pool,
                        tc.tile_pool(name="wts", bufs=1) as wts_pool,
                        tc.tile_pool(name="stageA", bufs=3) as stage_pool,
                        tc.tile_pool(name="psumA", bufs=4, space="PSUM") as psumA,
                    ):
                        for tt, (src, lw_d, b_d) in enumerate(
                            [(xs, lwx, bxv), (ys, lwy, byv)]
                        ):
                            lw = wts_pool.tile([96, 18, 128], BF16, tag=f"lw{tt}")
                            nc.sync.dma_start(
                                lw[:].rearrange("p (r k) m -> p r k m", k=3),
                                lw_d[:].transpose([2, 0, 1, 3]),
                            )
                            bias = wts_pool.tile([32, 1], F32, tag=f"bias{tt}")
                            nc.sync.dma_start(bias[:], b_d[:])

                            s_all = slab_pool.tile(
                                [96, D, 6, 50], BF16, tag="slab"
                            )
                            for kh in range(3):
                                nc.sync.dma_start(
                                    s_all[32 * kh : 32 * kh + 32],
                                    src[:, :, kh : kh + 6, :],
                                )

                            stages = {}
                            for b in range(9):
                                rels = [
                                    r for r in range(6) if 0 <= 4 * b + r - 1 < D
                                ]
                                pt = psumA.tile([128, 288], F32, tag="pa")
                                n_mm = 3 * len(rels)
                                i = 0
                                for rel in rels:
                                    din = 4 * b + rel - 1
                                    for kw in range(3):
                                        nc.tensor.matmul(
                                            pt[:],
                                            lw[:, 3 * rel + kw, :],
                                            s_all[:, din, :, kw : kw + 48],
                                            start=(i == 0),
                                            stop=(i == n_mm - 1),
                                        )
                                        i += 1
                                for dd in range(4):
                                    d = 4 * b + dd
                                    ld, pd = d // 9, d % 9
                                    if ld not in stages:
                                        stages[ld] = stage_pool.tile(
                                            [32, 9, 9, 32], BF16,
                                            tag="st", name="stage",
                                        )
                                    # bias add + (lhw, pw)->(pw, lhw), ->bf16
                                    nc.scalar.activation(
                                        stages[ld][:, pd].transpose([0, 2, 1]),
                                        pt[
                                            32 * dd : 32 * dd + 32
                                        ].rearrange("p (l w) -> p l w", w=9),
                                        Ident,
                                        bias=bias[:],
                                    )
                                    if pd == 8:
                                        nc.sync.dma_start(
                                            a2a_in[tt][
                                                :, :, :, 32 * ld : 32 * ld + 32
                                            ],
                                            stages.pop(ld)[:],
                                        )
                            if "c" in phases:
                                nc.gpsimd.collective_compute(
                                    "AllToAll",
                                    mybir.AluOpType.bypass,
                                    replica_groups=[list(range(N_CORES))],
                                    ins=[a2a_in[tt].opt()],
                                    outs=[a2a_out[tt].opt()],
                                )

                # ---------------- Stage B ----------------
                if "b" in phases:
                    with (
                        tc.tile_pool(name="wtsB", bufs=1) as wtsB,
                        tc.tile_pool(name="sbB", bufs=2) as sbB,
                        tc.tile_pool(name="attst", bufs=3) as attst_pool,
                        tc.tile_pool(name="psumU", bufs=2, space="PSUM") as psumU,
                        tc.tile_pool(name="psumV", bufs=2, space="PSUM") as psumV,
                        tc.tile_pool(name="psumT", bufs=2, space="PSUM") as psumT,
                    ):
                        w1a_sb = wtsB.tile([81, 81], BF16, tag="w1a")
                        w1b_sb = wtsB.tile([81, 81], BF16, tag="w1b")
                        w2a_sb = wtsB.tile([81, 81], BF16, tag="w2a")
                        w2b_sb = wtsB.tile([81, 81], BF16, tag="w2b")
                        nc.sync.dma_start(w1a_sb[:], w1ta[:])
                        nc.sync.dma_start(w1b_sb[:], w1tb[:])
                        nc.sync.dma_start(w2a_sb[:], w2t1[:])
                        nc.sync.dma_start(w2b_sb[:], w2t2[:])

                        for c_lo in range(4):
                            aT = []
                            for tt in range(2):
                                tT = sbB.tile([81, L], BF16, tag=f"tT{tt}")
                                nc.sync.dma_start(
                                    tT[:].rearrange(
                                        "p (a i c) -> p a i c", i=8, c=32
                                    ),
                                    a2a_out[tt][:, c_lo].rearrange(
                                        "i p (a c) -> p a i c", c=32
                                    ),
                                )
                                a_sb = sbB.tile([81, L], BF16, tag=f"aT{tt}")
                                for nch in range(2):
                                    sl = slice(512 * nch, 512 * nch + 512)
                                    uA = psumU.tile([81, 512], F32, tag="uA")
                                    uB = psumU.tile([81, 512], F32, tag="uB")
                                    nc.tensor.matmul(
                                        uA[:], w1a_sb[:], tT[:, sl],
                                        start=True, stop=True,
                                    )
                                    nc.tensor.matmul(
                                        uB[:], w1b_sb[:], tT[:, sl],
                                        start=True, stop=True,
                                    )
                                    uA_sb = sbB.tile([81, 512], BF16, tag="uAs")
                                    uB_sb = sbB.tile([81, 512], BF16, tag="uBs")
                                    nc.scalar.activation(uA_sb[:], uA[:], Copy)
                                    nc.vector.tensor_copy(uB_sb[:], uB[:])
                                    v = psumV.tile([81, 512], F32, tag="v")
                                    nc.tensor.matmul(
                                        v[:], w2a_sb[:], uA_sb[:],
                                        start=True, stop=False,
                                    )
                                    nc.tensor.matmul(
                                        v[:], w2b_sb[:], uB_sb[:],
                                        start=False, stop=True,
                                    )
                                    # leaky relu: max(0.2 v, v); only one PSUM
                                    # input per op -> stage v in SBUF first
                                    v_sb = sbB.tile([81, 512], BF16, tag="vs")
                                    nc.scalar.activation(v_sb[:], v[:], Copy)
                                    nc.vector.scalar_tensor_tensor(
                                        a_sb[:, sl], v_sb[:], 0.2, v[:],
                                        mult, amax,
                                    )
                                aT.append(a_sb)
                            aTx, aTy = aT
                            for lc in range(8):
                                st = attst_pool.tile([128, L], F32, tag="attst")
                                for nch in range(2):
                                    pa = psumT.tile([128, 512], F32, tag="pt")
                                    nc.tensor.matmul(
                                        pa[:],
                                        aTx[:, 128 * lc : 128 * lc + 128],
                                        aTy[:, 512 * nch : 512 * nch + 512],
                                        start=True, stop=True,
                                    )
                                    dst = st[:, 512 * nch : 512 * nch + 512]
                                    if nch == 0:
                                        nc.scalar.activation(dst, pa[:], Copy)
                                    else:
                                        nc.vector.tensor_copy(dst, pa[:])
                                nc.sync.dma_start(
                                    att[c_lo, 128 * lc : 128 * lc + 128, :],
                                    st[:],
                                )

    nc.compile()
    return nc


def host_inputs(x, y, Wx, bx, Wy, by, W1, W2):
    x = np.asarray(x, np.float32)
    y = np.asarray(y, np.float32)
    Wx = np.asarray(Wx, np.float32)
    bx = np.asarray(bx, np.float32)
    Wy = np.asarray(Wy, np.float32)
    by = np.asarray(by, np.float32)
    W1 = np.asarray(W1, np.float32)
    W2 = np.asarray(W2, np.float32)

    def strips(x0):
        out = []
        for j in range(N_CORES):
            s = np.zeros((C, D, 8, 50), np.float32)
            r0, r1 = max(0, 6 * j - 1), min(48, 6 * j + 7)
            d0 = r0 - (6 * j - 1)
            s[:, :, d0 : d0 + (r1 - r0), 1:49] = x0[:, :, r0:r1, :]
            out.append(s.astype(BF_NP))
        return out

    def make_lw(Wc):
        # lw[rel, kw, kh*32+ci, 32*dd+co] = Wc[co, ci, rel-dd, kh, kw]
        lw = np.zeros((6, 3, 96, 128), np.float32)
        for rel in range(6):
            for dd in range(4):
                kd = rel - dd
                if 0 <= kd < 3:
                    # (co, ci, kh, kw) -> (kw, kh, ci, co)
                    blk = np.transpose(Wc[:, :, kd], (3, 2, 1, 0)).reshape(
                        3, 96, 32
                    )
                    lw[rel, :, :, 32 * dd : 32 * dd + 32] = blk
        return lw.astype(BF_NP)

    xs_l, ys_l = strips(x[0]), strips(y[0])
    common = {
        "lwx": make_lw(Wx),
        "lwy": make_lw(Wy),
        "bxv": np.ascontiguousarray(bx[:, None]),
        "byv": np.ascontiguousarray(by[:, None]),
        "w1ta": np.ascontiguousarray(W1[:81].T).astype(BF_NP),
        "w1tb": np.ascontiguousarray(W1[81:].T).astype(BF_NP),
        "w2t1": np.ascontiguousarray((W2 / 9.0)[:, :81].T).astype(BF_NP),
        "w2t2": np.ascontiguousarray((W2 / 9.0)[:, 81:].T).astype(BF_NP),
    }
    return [
        {"xs": xs_l[j], "ys": ys_l[j], **common} for j in range(N_CORES)
    ]


_CACHED_NC = None


def get_program():
    global _CACHED_NC
    if _CACHED_NC is None:
        _CACHED_NC = build_program()
    return _CACHED_NC


def kernel(x, y, Wx, bx, Wy, by, W1, W2):
    nc = get_program()
    in_maps = host_inputs(x, y, Wx, bx, Wy, by, W1, W2)
    res = bass_utils.run_bass_kernel_spmd(
        nc, in_maps, core_ids=list(range(N_CORES))
    )
    out = np.concatenate([r["att"] for r in res.results], axis=0)[None]
    return out
